# revision 32
# baseline (speedup 1.0000x reference)
"""Trainium2 Bass kernel for the SOCNet battery state-of-charge model.

Math (per battery cell b, timestep t):
    h   = softplus(w0*I + w1*Temp + b1e)
    f   = eta0*(1 + w2e*h + b2e) * I / (3600*Q)
    out[b, 0] = SOC_init(b)          (tiny net on first-timestep features)
    out[b, t] = SOC_init(b) + sum_{j<t} (ts[j+1]-ts[j]) * f[j]

Strategy: pure data parallel over 8 NeuronCores (128 batch rows per core =
128 SBUF partitions).  The tiny per-cell scalars (SOC_init, q1, q2) are
precomputed on host.  Streamed inputs are staged quantization-style: time
column f32 (dt needs full precision), and the eta-net inputs as bf16
scaled streams i = w0*I, m = w1*Temp with 1/w0 folded into q1/q2 so the
i-stream feeds both the pre-activation and the dt*I product.  The SOC
output is written bf16 (scan state stays fp32 in hardware) and upcast on
host.  Per-core traffic: 21 MB -> 12.6 MB.

The kernel is DVE-bound (~1 elem/cycle at 0.96 GHz; the 16-bit fast modes
do not materialize on HW and gpsimd tensor ops are far slower).  Per chunk
(TC=4096) the work splits so the DVE runs only 4 passes:
    DVE    : dt diff, dt*i, *w, scan
    ACT    : exp, ln, affine q2*h+q1 (one act table), out-DMA doorbell
    gpsimd : SWDGE accum-add DMA assembles the pre-act a = i + m straight
             from DRAM (<=2048-element pieces: larger accums wedge the HW)
    sync   : input DMA doorbells (HWDGE queue)
The carry chains through the scan's own bf16 last column (<=0.2% of the
value, at a single chunk boundary).
"""

import numpy as np

B, T, F = 1024, 8192, 4
NCORES = 8
BS = B // NCORES  # 128 rows per core == SBUF partition count
TC = 2048         # timesteps per chunk


def _softplus64(x):
    x = x.astype(np.float64)
    return np.logaddexp(0.0, x)


def _build_program(fold, w0=0.0, reps=1):
    """fold=True: the 'i' stream is host-staged w0*I (bf16) and 1/w0 is
    folded into the per-cell q1/q2, so one stream feeds both the pre-act
    (via a gpsimd accum-add DMA: a = i + tp, no DVE pass) and the dt*I
    product.  fold=False (|w0| ~ 0 fallback): 'i' is raw I and the pre-act
    is assembled on the DVE with an extra stt pass."""
    from contextlib import ExitStack

    import bass_rust as _bass_rust
    import concourse.bass as bass
    import concourse.mybir as mybir
    import concourse.tile as tile

    f32 = mybir.dt.float32
    f8 = mybir.dt.float8e4
    bf16 = mybir.dt.bfloat16
    nc = bass.Bass()

    td = nc.dram_tensor("t", [BS, T], f32, kind="ExternalInput")
    id_ = nc.dram_tensor("i", [BS, T], bf16, kind="ExternalInput")   # w0*I
    md = nc.dram_tensor("m", [BS, T], bf16, kind="ExternalInput")    # w1*Temp
    pd = nc.dram_tensor("p", [BS, 4], f32, kind="ExternalInput")
    od = nc.dram_tensor("o", [BS, T], bf16, kind="ExternalOutput")

    with ExitStack() as ctx:
        tc = ctx.enter_context(tile.TileContext(nc))
        tpool = ctx.enter_context(tc.tile_pool(name="t", bufs=3))
        ipool = ctx.enter_context(tc.tile_pool(name="i", bufs=4))
        apool = ctx.enter_context(tc.tile_pool(name="a", bufs=4))
        wpool = ctx.enter_context(tc.tile_pool(name="w", bufs=3))
        mpool = ctx.enter_context(tc.tile_pool(name="m", bufs=3))
        rpool = ctx.enter_context(tc.tile_pool(name="r", bufs=4))
        cpool = ctx.enter_context(tc.tile_pool(name="c", bufs=1))

        ones = cpool.tile([BS, TC], f32)
        nc.vector.memset(ones[:], 1.0)
        ptile = cpool.tile([BS, 4], f32)
        nc.sync.dma_start(ptile[:], pd[:])
        # DVE-made copy of the per-cell scalars: the activations' bias/scale
        # reads then depend only on the DVE semaphore (the Activation ISA
        # struct has a single sync-wait slot, and every activation here
        # already waits on a DVE-produced input).
        pact = cpool.tile([BS, 4], f32)
        nc.vector.tensor_copy(pact[:], ptile[:])

        sizes = []
        rem = T - 1
        while rem > 0:
            sizes.append(min(TC, rem))
            rem -= sizes[-1]

        for _rep in range(reps):
            carry = ptile[:, 0:1]  # SOC_init
            s = 0
            for L in sizes:
                tt = tpool.tile([BS, TC + 1], f32)
                nc.sync.dma_start(tt[:, : L + 1], td[:, s : s + L + 1])
                it = ipool.tile([BS, TC], bf16)
                nc.sync.dma_start(it[:, :L], id_[:, s : s + L])
                at = apool.tile([BS, TC], bf16)
                nc.sync.dma_start(at[:, :L], md[:, s : s + L])
                if fold:
                    # a = (w0*I) + (w1*Temp): assembled by the DMA subsystem
                    # (gpsimd SWDGE accum-add, sourced straight from DRAM so
                    # it depends only on at's base load) instead of a DVE
                    # pass — the DVE is this kernel's critical resource.
                    # SWDGE accum wedges the device above 4KB/partition, so
                    # issue it in <=2048-element pieces.
                    for p0 in range(0, L, 2048):
                        pL = min(2048, L - p0)
                        nc.gpsimd.dma_start(
                            at[:, p0 : p0 + pL], id_[:, s + p0 : s + p0 + pL],
                            accum_op=mybir.AluOpType.add,
                        )
                    ain = at[:, :L]
                else:
                    # rare path (|w0| ~ 0): minimal SBUF footprint
                    aw = mpool.tile([BS, TC], bf16, name="aw", bufs=1)
                    nc.vector.scalar_tensor_tensor(
                        aw[:, :L], it[:, :L], float(w0), at[:, :L],
                        mybir.AluOpType.mult, mybir.AluOpType.add,
                    )
                    ain = aw[:, :L]

                # wt = softplus(a + b1e) = ln(1 + exp(.)); f32 intermediates
                # (bf16 DVE tiles measured slower: no 16-bit fast modes here)
                wt = wpool.tile([BS, TC], f32)
                nc.scalar.activation(
                    wt[:, :L], ain, mybir.ActivationFunctionType.Exp,
                    bias=pact[:, 3:4], scale=1.0,
                )
                nc.scalar.activation(
                    wt[:, :L], wt[:, :L], mybir.ActivationFunctionType.Ln,
                    bias=1.0, scale=1.0,
                )
                # wt = q2*wt + q1    (per-cell scalars; q's carry the 1/w0)
                nc.scalar.activation(
                    wt[:, :L], wt[:, :L], mybir.ActivationFunctionType.Identity,
                    bias=pact[:, 1:2], scale=pact[:, 2:3],
                )
                # mt = dt * (w0*I), then in-place mt = mt * wt
                mt = mpool.tile([BS, TC], f32)
                nc.vector.tensor_sub(mt[:, :L], tt[:, 1 : L + 1], tt[:, 0:L])
                nc.vector.tensor_tensor(
                    mt[:, :L], mt[:, :L], it[:, :L], mybir.AluOpType.mult
                )
                nc.vector.tensor_tensor(
                    mt[:, :L], mt[:, :L], wt[:, :L], mybir.AluOpType.mult
                )
                # running SOC: r[i] = carry + cumsum(mt)[i], stored bf16.
                # Carry chains through the scan's own last column (bf16 → at
                # most ~0.2% of the value per chunk boundary, 1 boundary).
                rt = rpool.tile([BS, TC], bf16)
                nc.vector.tensor_tensor_scan(
                    rt[:, :L], ones[:, :L], mt[:, :L], carry,
                    mybir.AluOpType.mult, mybir.AluOpType.add,
                )
                nc.scalar.dma_start(od[:, s + 1 : s + L + 1], rt[:, :L])
                carry = rt[:, L - 1 : L]
                s += L

    # neuronxcc codegen allows at most one sync wait per instruction; split
    # multi-wait instructions the way Bacc.compile() would.
    _bass_rust.generate_event_semaphores(nc)
    return nc


def _prep(inputs):
    """Host-side prep shared by kernel() and the bench harness: per-cell
    scalar precompute + per-core input staging (slice, cast, pack).
    Returns (params, in_maps, soc_init) with params matching
    _build_program's signature."""
    import ml_dtypes

    X = np.ascontiguousarray(np.asarray(inputs["X"]), dtype=np.float32)
    SC = np.ascontiguousarray(np.asarray(inputs["SC"]), dtype=np.float32)
    W1i = np.asarray(inputs["W1i"], dtype=np.float64)
    b1i = np.asarray(inputs["b1i"], dtype=np.float64)
    W2i = np.asarray(inputs["W2i"], dtype=np.float64)
    b2i = np.asarray(inputs["b2i"], dtype=np.float64)
    W1e = np.asarray(inputs["W1e"], dtype=np.float64)
    b1e = np.asarray(inputs["b1e"], dtype=np.float64)
    W2e = np.asarray(inputs["W2e"], dtype=np.float64)
    b2e = np.asarray(inputs["b2e"], dtype=np.float64)

    # ---- host precompute of tiny per-cell scalars (float64 for accuracy) ----
    Q = SC[:, 0].astype(np.float64)
    eta0 = SC[:, 1].astype(np.float64)
    soc_base = SC[:, 3].astype(np.float64)

    feat0 = np.stack(
        [X[:, 0, 1], X[:, 0, 2], X[:, 0, 3], SC[:, 2]], axis=-1
    ).astype(np.float64)  # [B, 4] = (I0, Temp0, U0, R)
    z = feat0 @ W1i.T + b1i
    h0 = _softplus64(z)
    soc_net = (h0 @ W2i.T + b2i)[:, 0]
    soc_init = (soc_base * (1.0 + soc_net)).astype(np.float32)  # [B]

    c = eta0 / (3600.0 * Q)
    b2e_f = float(np.asarray(b2e).reshape(-1)[0])
    w2e_f = float(np.asarray(W2e).reshape(-1)[0])
    q1 = c * (1.0 + b2e_f)  # [B]
    q2 = c * w2e_f          # [B]

    # Stage the eta-net inputs quantization-style: i-stream = w0*I, m-stream
    # = w1*Temp (bf16, scale folded into the cast).  With fold=True, 1/w0
    # moves into q1/q2 so the same i-stream also serves the dt*I product:
    #   incr = dt*I*(q1+q2*h) = dt*(w0*I)*((q1+q2*h)/w0)
    w0 = float(np.asarray(W1e).reshape(-1)[0])
    w1 = float(np.asarray(W1e).reshape(-1)[1])
    b1e_f = float(np.asarray(b1e).reshape(-1)[0])
    fold = abs(w0) > 1e-20
    if fold:
        q1 = q1 / w0
        q2 = q2 / w0
        iscale = w0
    else:
        iscale = 1.0  # raw I stream; pre-act assembled on DVE with w0=0

    P = np.stack(
        [soc_init.astype(np.float64), q1, q2, np.full_like(q1, b1e_f)], axis=-1
    ).astype(np.float32)  # [B, 4]

    tcol = np.ascontiguousarray(X[:, :, 0])                           # [B, T] f32
    icol = (X[:, :, 1] * np.float32(iscale)).astype(ml_dtypes.bfloat16)
    mcol = (X[:, :, 2] * np.float32(w1)).astype(ml_dtypes.bfloat16)

    in_maps = []
    for ci in range(NCORES):
        sl = slice(ci * BS, (ci + 1) * BS)
        in_maps.append(
            {
                "t": tcol[sl],
                "i": np.ascontiguousarray(icol[sl]),
                "m": np.ascontiguousarray(mcol[sl]),
                "p": np.ascontiguousarray(P[sl]),
            }
        )

    return (fold, w0), in_maps, soc_init


def kernel(X, SC, W1i, b1i, W2i, b2i, W1e, b1e, W2e, b2e):
    from concourse.bass_utils import run_bass_kernel_spmd

    params, in_maps, soc_init = _prep(
        dict(X=X, SC=SC, W1i=W1i, b1i=b1i, W2i=W2i, b2i=b2i,
             W1e=W1e, b1e=b1e, W2e=W2e, b2e=b2e)
    )
    nc = _build_program(*params)

    res = run_bass_kernel_spmd(nc, in_maps, list(range(NCORES)))
    out = np.concatenate(
        [res.results[ci]["o"].astype(np.float32) for ci in range(NCORES)], axis=0
    )
    out[:, 0] = soc_init  # device never writes column 0
    return out.reshape(B, T, 1)


# revision 33
# speedup vs baseline: 1.0897x; 1.0897x over previous
"""Trainium2 Bass kernel for the SOCNet battery state-of-charge model.

Math (per battery cell b, timestep t):
    h   = softplus(w0*I + w1*Temp + b1e)
    f   = eta0*(1 + w2e*h + b2e) * I / (3600*Q)
    out[b, 0] = SOC_init(b)          (tiny net on first-timestep features)
    out[b, t] = SOC_init(b) + sum_{j<t} (ts[j+1]-ts[j]) * f[j]

Strategy: pure data parallel over 8 NeuronCores (128 batch rows per core =
128 SBUF partitions).  The tiny per-cell scalars (SOC_init, q1, q2) are
precomputed on host.  Streamed inputs are staged quantization-style: time
column f32 (dt needs full precision), and the eta-net inputs as bf16
scaled streams i = w0*I, m = w1*Temp with 1/w0 folded into q1/q2 so the
i-stream feeds both the pre-activation and the dt*I product.  The SOC
output is written bf16 (scan state stays fp32 in hardware) and upcast on
host.  Per-core traffic: 21 MB -> 12.6 MB.

The kernel is DVE-bound (~1 elem/cycle at 0.96 GHz; the 16-bit fast modes
do not materialize on HW and gpsimd tensor ops are far slower).  Per chunk
(TC=4096) the work splits so the DVE runs only 4 passes:
    DVE    : dt diff, dt*i, *w, scan
    ACT    : exp, ln, affine q2*h+q1 (one act table), out-DMA doorbell
    gpsimd : SWDGE accum-add DMA assembles the pre-act a = i + m straight
             from DRAM (<=2048-element pieces: larger accums wedge the HW)
    sync   : input DMA doorbells (HWDGE queue)
The carry chains through the scan's own bf16 last column (<=0.2% of the
value, at a single chunk boundary).
"""

import numpy as np

B, T, F = 1024, 8192, 4
NCORES = 8
BS = B // NCORES  # 128 rows per core == SBUF partition count
TC = 4096         # timesteps per chunk


def _softplus64(x):
    x = x.astype(np.float64)
    return np.logaddexp(0.0, x)


def _build_program(fold, w0=0.0, reps=1):
    """fold=True: the 'i' stream is host-staged w0*I (bf16) and 1/w0 is
    folded into the per-cell q1/q2, so one stream feeds both the pre-act
    (via a gpsimd accum-add DMA: a = i + tp, no DVE pass) and the dt*I
    product.  fold=False (|w0| ~ 0 fallback): 'i' is raw I and the pre-act
    is assembled on the DVE with an extra stt pass."""
    from contextlib import ExitStack

    import bass_rust as _bass_rust
    import concourse.bass as bass
    import concourse.mybir as mybir
    import concourse.tile as tile

    f32 = mybir.dt.float32
    f8 = mybir.dt.float8e4
    bf16 = mybir.dt.bfloat16
    nc = bass.Bass()

    td = nc.dram_tensor("t", [BS, T], f32, kind="ExternalInput")
    id_ = nc.dram_tensor("i", [BS, T], bf16, kind="ExternalInput")   # w0*I
    md = nc.dram_tensor("m", [BS, T], bf16, kind="ExternalInput")    # w1*Temp
    pd = nc.dram_tensor("p", [BS, 4], f32, kind="ExternalInput")
    od = nc.dram_tensor("o", [BS, T], bf16, kind="ExternalOutput")

    with ExitStack() as ctx:
        tc = ctx.enter_context(tile.TileContext(nc))
        tpool = ctx.enter_context(tc.tile_pool(name="t", bufs=2))
        ipool = ctx.enter_context(tc.tile_pool(name="i", bufs=3))
        apool = ctx.enter_context(tc.tile_pool(name="a", bufs=3))
        wpool = ctx.enter_context(tc.tile_pool(name="w", bufs=2))
        mpool = ctx.enter_context(tc.tile_pool(name="m", bufs=2))
        rpool = ctx.enter_context(tc.tile_pool(name="r", bufs=3))
        cpool = ctx.enter_context(tc.tile_pool(name="c", bufs=1))

        ones = cpool.tile([BS, TC], f32)
        nc.vector.memset(ones[:], 1.0)
        ptile = cpool.tile([BS, 4], f32)
        nc.sync.dma_start(ptile[:], pd[:])
        # DVE-made copy of the per-cell scalars: the activations' bias/scale
        # reads then depend only on the DVE semaphore (the Activation ISA
        # struct has a single sync-wait slot, and every activation here
        # already waits on a DVE-produced input).
        pact = cpool.tile([BS, 4], f32)
        nc.vector.tensor_copy(pact[:], ptile[:])

        sizes = []
        rem = T - 1
        while rem > 0:
            sizes.append(min(TC, rem))
            rem -= sizes[-1]

        for _rep in range(reps):
            carry = ptile[:, 0:1]  # SOC_init
            s = 0
            for L in sizes:
                tt = tpool.tile([BS, TC + 1], f32)
                nc.sync.dma_start(tt[:, : L + 1], td[:, s : s + L + 1])
                it = ipool.tile([BS, TC], bf16)
                nc.sync.dma_start(it[:, :L], id_[:, s : s + L])
                at = apool.tile([BS, TC], bf16)
                nc.sync.dma_start(at[:, :L], md[:, s : s + L])
                if fold:
                    # a = (w0*I) + (w1*Temp): assembled by the DMA subsystem
                    # (gpsimd SWDGE accum-add, sourced straight from DRAM so
                    # it depends only on at's base load) instead of a DVE
                    # pass — the DVE is this kernel's critical resource.
                    # SWDGE accum wedges the device above 4KB/partition, so
                    # issue it in <=2048-element pieces.
                    for p0 in range(0, L, 2048):
                        pL = min(2048, L - p0)
                        nc.gpsimd.dma_start(
                            at[:, p0 : p0 + pL], id_[:, s + p0 : s + p0 + pL],
                            accum_op=mybir.AluOpType.add,
                        )
                    ain = at[:, :L]
                else:
                    # rare path (|w0| ~ 0): minimal SBUF footprint
                    aw = mpool.tile([BS, TC], bf16, name="aw", bufs=1)
                    nc.vector.scalar_tensor_tensor(
                        aw[:, :L], it[:, :L], float(w0), at[:, :L],
                        mybir.AluOpType.mult, mybir.AluOpType.add,
                    )
                    ain = aw[:, :L]

                # wt = softplus(a + b1e) = ln(1 + exp(.)); f32 intermediates
                # (bf16 DVE tiles measured slower: no 16-bit fast modes here)
                wt = wpool.tile([BS, TC], f32)
                nc.scalar.activation(
                    wt[:, :L], ain, mybir.ActivationFunctionType.Exp,
                    bias=pact[:, 3:4], scale=1.0,
                )
                nc.scalar.activation(
                    wt[:, :L], wt[:, :L], mybir.ActivationFunctionType.Ln,
                    bias=1.0, scale=1.0,
                )
                # wt = q2*wt + q1    (per-cell scalars; q's carry the 1/w0)
                nc.scalar.activation(
                    wt[:, :L], wt[:, :L], mybir.ActivationFunctionType.Identity,
                    bias=pact[:, 1:2], scale=pact[:, 2:3],
                )
                # mt = dt * (w0*I), then in-place mt = mt * wt
                mt = mpool.tile([BS, TC], f32)
                nc.vector.tensor_sub(mt[:, :L], tt[:, 1 : L + 1], tt[:, 0:L])
                nc.vector.tensor_tensor(
                    mt[:, :L], mt[:, :L], it[:, :L], mybir.AluOpType.mult
                )
                nc.vector.tensor_tensor(
                    mt[:, :L], mt[:, :L], wt[:, :L], mybir.AluOpType.mult
                )
                # running SOC: r[i] = carry + cumsum(mt)[i], stored bf16.
                # Carry chains through the scan's own last column (bf16 → at
                # most ~0.2% of the value per chunk boundary, 1 boundary).
                rt = rpool.tile([BS, TC], bf16)
                nc.vector.tensor_tensor_scan(
                    rt[:, :L], ones[:, :L], mt[:, :L], carry,
                    mybir.AluOpType.mult, mybir.AluOpType.add,
                )
                nc.scalar.dma_start(od[:, s + 1 : s + L + 1], rt[:, :L])
                carry = rt[:, L - 1 : L]
                s += L

    # neuronxcc codegen allows at most one sync wait per instruction; split
    # multi-wait instructions the way Bacc.compile() would.
    _bass_rust.generate_event_semaphores(nc)
    return nc


def _prep(inputs):
    """Host-side prep shared by kernel() and the bench harness: per-cell
    scalar precompute + per-core input staging (slice, cast, pack).
    Returns (params, in_maps, soc_init) with params matching
    _build_program's signature."""
    import ml_dtypes

    X = np.ascontiguousarray(np.asarray(inputs["X"]), dtype=np.float32)
    SC = np.ascontiguousarray(np.asarray(inputs["SC"]), dtype=np.float32)
    W1i = np.asarray(inputs["W1i"], dtype=np.float64)
    b1i = np.asarray(inputs["b1i"], dtype=np.float64)
    W2i = np.asarray(inputs["W2i"], dtype=np.float64)
    b2i = np.asarray(inputs["b2i"], dtype=np.float64)
    W1e = np.asarray(inputs["W1e"], dtype=np.float64)
    b1e = np.asarray(inputs["b1e"], dtype=np.float64)
    W2e = np.asarray(inputs["W2e"], dtype=np.float64)
    b2e = np.asarray(inputs["b2e"], dtype=np.float64)

    # ---- host precompute of tiny per-cell scalars (float64 for accuracy) ----
    Q = SC[:, 0].astype(np.float64)
    eta0 = SC[:, 1].astype(np.float64)
    soc_base = SC[:, 3].astype(np.float64)

    feat0 = np.stack(
        [X[:, 0, 1], X[:, 0, 2], X[:, 0, 3], SC[:, 2]], axis=-1
    ).astype(np.float64)  # [B, 4] = (I0, Temp0, U0, R)
    z = feat0 @ W1i.T + b1i
    h0 = _softplus64(z)
    soc_net = (h0 @ W2i.T + b2i)[:, 0]
    soc_init = (soc_base * (1.0 + soc_net)).astype(np.float32)  # [B]

    c = eta0 / (3600.0 * Q)
    b2e_f = float(np.asarray(b2e).reshape(-1)[0])
    w2e_f = float(np.asarray(W2e).reshape(-1)[0])
    q1 = c * (1.0 + b2e_f)  # [B]
    q2 = c * w2e_f          # [B]

    # Stage the eta-net inputs quantization-style: i-stream = w0*I, m-stream
    # = w1*Temp (bf16, scale folded into the cast).  With fold=True, 1/w0
    # moves into q1/q2 so the same i-stream also serves the dt*I product:
    #   incr = dt*I*(q1+q2*h) = dt*(w0*I)*((q1+q2*h)/w0)
    w0 = float(np.asarray(W1e).reshape(-1)[0])
    w1 = float(np.asarray(W1e).reshape(-1)[1])
    b1e_f = float(np.asarray(b1e).reshape(-1)[0])
    fold = abs(w0) > 1e-20
    if fold:
        q1 = q1 / w0
        q2 = q2 / w0
        iscale = w0
    else:
        iscale = 1.0  # raw I stream; pre-act assembled on DVE with w0=0

    P = np.stack(
        [soc_init.astype(np.float64), q1, q2, np.full_like(q1, b1e_f)], axis=-1
    ).astype(np.float32)  # [B, 4]

    tcol = np.ascontiguousarray(X[:, :, 0])                           # [B, T] f32
    icol = (X[:, :, 1] * np.float32(iscale)).astype(ml_dtypes.bfloat16)
    mcol = (X[:, :, 2] * np.float32(w1)).astype(ml_dtypes.bfloat16)

    in_maps = []
    for ci in range(NCORES):
        sl = slice(ci * BS, (ci + 1) * BS)
        in_maps.append(
            {
                "t": tcol[sl],
                "i": np.ascontiguousarray(icol[sl]),
                "m": np.ascontiguousarray(mcol[sl]),
                "p": np.ascontiguousarray(P[sl]),
            }
        )

    return (fold, w0), in_maps, soc_init


def kernel(X, SC, W1i, b1i, W2i, b2i, W1e, b1e, W2e, b2e):
    from concourse.bass_utils import run_bass_kernel_spmd

    params, in_maps, soc_init = _prep(
        dict(X=X, SC=SC, W1i=W1i, b1i=b1i, W2i=W2i, b2i=b2i,
             W1e=W1e, b1e=b1e, W2e=W2e, b2e=b2e)
    )
    nc = _build_program(*params)

    res = run_bass_kernel_spmd(nc, in_maps, list(range(NCORES)))
    out = np.concatenate(
        [res.results[ci]["o"].astype(np.float32) for ci in range(NCORES)], axis=0
    )
    out[:, 0] = soc_init  # device never writes column 0
    return out.reshape(B, T, 1)


# revision 34
# speedup vs baseline: 1.1182x; 1.0261x over previous
"""Trainium2 Bass kernel for the SOCNet battery state-of-charge model.

Math (per battery cell b, timestep t):
    h   = softplus(w0*I + w1*Temp + b1e)
    f   = eta0*(1 + w2e*h + b2e) * I / (3600*Q)
    out[b, 0] = SOC_init(b)          (tiny net on first-timestep features)
    out[b, t] = SOC_init(b) + sum_{j<t} (ts[j+1]-ts[j]) * f[j]

Strategy: pure data parallel over 8 NeuronCores (128 batch rows per core =
128 SBUF partitions).  The tiny per-cell scalars (SOC_init, q1, q2) are
precomputed on host.  Streamed inputs are staged quantization-style: time
column f32 (dt needs full precision), and the eta-net inputs as bf16
scaled streams i = w0*I, m = w1*Temp with 1/w0 folded into q1/q2 so the
i-stream feeds both the pre-activation and the dt*I product.  The SOC
output is written bf16 (scan state stays fp32 in hardware) and upcast on
host.  Per-core traffic: 21 MB -> 12.6 MB.

The kernel is DVE-bound (~1 elem/cycle at 0.96 GHz; the 16-bit fast modes
do not materialize on HW and gpsimd tensor ops are far slower).  Per chunk
(TC=4096) the work splits so the DVE runs only 4 passes:
    DVE    : dt diff, dt*i, *w, scan
    ACT    : exp, ln, affine q2*h+q1 (one act table), out-DMA doorbell
    gpsimd : SWDGE accum-add DMA assembles the pre-act a = i + m straight
             from DRAM (<=2048-element pieces: larger accums wedge the HW)
    sync   : input DMA doorbells (HWDGE queue)
The carry chains through the scan's own bf16 last column (<=0.2% of the
value, at a single chunk boundary).
"""

import numpy as np

B, T, F = 1024, 8192, 4
NCORES = 8
BS = B // NCORES  # 128 rows per core == SBUF partition count
TC = 4096         # timesteps per chunk


def _softplus64(x):
    x = x.astype(np.float64)
    return np.logaddexp(0.0, x)


def _build_program(fold, w0=0.0, reps=1):
    """fold=True: the 'i' stream is host-staged w0*I (bf16) and 1/w0 is
    folded into the per-cell q1/q2, so one stream feeds both the pre-act
    (via a gpsimd accum-add DMA: a = i + tp, no DVE pass) and the dt*I
    product.  fold=False (|w0| ~ 0 fallback): 'i' is raw I and the pre-act
    is assembled on the DVE with an extra stt pass."""
    from contextlib import ExitStack

    import bass_rust as _bass_rust
    import concourse.bass as bass
    import concourse.mybir as mybir
    import concourse.tile as tile

    f32 = mybir.dt.float32
    f8 = mybir.dt.float8e4
    bf16 = mybir.dt.bfloat16
    nc = bass.Bass()

    td = nc.dram_tensor("t", [BS, T], f32, kind="ExternalInput")
    id_ = nc.dram_tensor("i", [BS, T], bf16, kind="ExternalInput")   # w0*I
    md = nc.dram_tensor("m", [BS, T], bf16, kind="ExternalInput")    # w1*Temp
    pd = nc.dram_tensor("p", [BS, 4], f32, kind="ExternalInput")
    od = nc.dram_tensor("o", [BS, T], bf16, kind="ExternalOutput")

    with ExitStack() as ctx:
        tc = ctx.enter_context(tile.TileContext(nc))
        tpool = ctx.enter_context(tc.tile_pool(name="t", bufs=2))
        ipool = ctx.enter_context(tc.tile_pool(name="i", bufs=3))
        apool = ctx.enter_context(tc.tile_pool(name="a", bufs=3))
        wpool = ctx.enter_context(tc.tile_pool(name="w", bufs=2))
        mpool = ctx.enter_context(tc.tile_pool(name="m", bufs=2))
        rpool = ctx.enter_context(tc.tile_pool(name="r", bufs=3))
        cpool = ctx.enter_context(tc.tile_pool(name="c", bufs=1))

        ones = cpool.tile([BS, TC], f32)
        nc.vector.memset(ones[:], 1.0)
        ptile = cpool.tile([BS, 4], f32)
        nc.sync.dma_start(ptile[:], pd[:])
        # DVE-made copy of the per-cell scalars: the activations' bias/scale
        # reads then depend only on the DVE semaphore (the Activation ISA
        # struct has a single sync-wait slot, and every activation here
        # already waits on a DVE-produced input).
        pact = cpool.tile([BS, 4], f32)
        nc.vector.tensor_copy(pact[:], ptile[:])

        sizes = []
        rem = T - 1
        while rem > 0:
            sizes.append(min(TC, rem))
            rem -= sizes[-1]

        for _rep in range(reps):
            carry = ptile[:, 0:1]  # SOC_init
            s = 0
            for L in sizes:
                tt = tpool.tile([BS, TC + 1], f32)
                nc.sync.dma_start(tt[:, : L + 1], td[:, s : s + L + 1])
                it = ipool.tile([BS, TC], bf16)
                nc.sync.dma_start(it[:, :L], id_[:, s : s + L])
                at = apool.tile([BS, TC], bf16)
                nc.sync.dma_start(at[:, :L], md[:, s : s + L])
                if fold:
                    # a = (w0*I) + (w1*Temp): assembled by the DMA subsystem
                    # (gpsimd SWDGE accum-add, sourced straight from DRAM so
                    # it depends only on at's base load) instead of a DVE
                    # pass — the DVE is this kernel's critical resource.
                    # SWDGE accum wedges the device above 4KB/partition, so
                    # issue it in <=2048-element pieces.
                    for p0 in range(0, L, 2048):
                        pL = min(2048, L - p0)
                        nc.gpsimd.dma_start(
                            at[:, p0 : p0 + pL], id_[:, s + p0 : s + p0 + pL],
                            accum_op=mybir.AluOpType.add,
                        )
                    ain = at[:, :L]
                else:
                    # rare path (|w0| ~ 0): minimal SBUF footprint
                    aw = mpool.tile([BS, TC], bf16, name="aw", bufs=1)
                    nc.vector.scalar_tensor_tensor(
                        aw[:, :L], it[:, :L], float(w0), at[:, :L],
                        mybir.AluOpType.mult, mybir.AluOpType.add,
                    )
                    ain = aw[:, :L]

                # wt = softplus(a + b1e) = ln(1 + exp(.)); f32 intermediates
                # (bf16 DVE tiles measured slower: no 16-bit fast modes here)
                wt = wpool.tile([BS, TC], f32)
                nc.scalar.activation(
                    wt[:, :L], ain, mybir.ActivationFunctionType.Exp,
                    bias=pact[:, 3:4], scale=1.0,
                )
                nc.scalar.activation(
                    wt[:, :L], wt[:, :L], mybir.ActivationFunctionType.Ln,
                    bias=1.0, scale=1.0,
                )
                # wt = q2*wt + q1    (per-cell scalars; q's carry the 1/w0)
                nc.scalar.activation(
                    wt[:, :L], wt[:, :L], mybir.ActivationFunctionType.Identity,
                    bias=pact[:, 1:2], scale=pact[:, 2:3],
                )
                # mt = dt * (w0*I), then in-place mt = mt * wt
                mt = mpool.tile([BS, TC], f32)
                nc.vector.tensor_sub(mt[:, :L], tt[:, 1 : L + 1], tt[:, 0:L])
                nc.vector.tensor_tensor(
                    mt[:, :L], mt[:, :L], it[:, :L], mybir.AluOpType.mult
                )
                nc.vector.tensor_tensor(
                    mt[:, :L], mt[:, :L], wt[:, :L], mybir.AluOpType.mult
                )
                # running SOC: r[i] = carry + cumsum(mt)[i], stored bf16.
                # Carry chains through the scan's own last column (bf16 → at
                # most ~0.2% of the value per chunk boundary, 1 boundary).
                rt = rpool.tile([BS, TC], bf16)
                nc.vector.tensor_tensor_scan(
                    rt[:, :L], ones[:, :L], mt[:, :L], carry,
                    mybir.AluOpType.mult, mybir.AluOpType.add,
                )
                nc.sync.dma_start(od[:, s + 1 : s + L + 1], rt[:, :L])
                carry = rt[:, L - 1 : L]
                s += L

    # neuronxcc codegen allows at most one sync wait per instruction; split
    # multi-wait instructions the way Bacc.compile() would.
    _bass_rust.generate_event_semaphores(nc)
    return nc


def _prep(inputs):
    """Host-side prep shared by kernel() and the bench harness: per-cell
    scalar precompute + per-core input staging (slice, cast, pack).
    Returns (params, in_maps, soc_init) with params matching
    _build_program's signature."""
    import ml_dtypes

    X = np.ascontiguousarray(np.asarray(inputs["X"]), dtype=np.float32)
    SC = np.ascontiguousarray(np.asarray(inputs["SC"]), dtype=np.float32)
    W1i = np.asarray(inputs["W1i"], dtype=np.float64)
    b1i = np.asarray(inputs["b1i"], dtype=np.float64)
    W2i = np.asarray(inputs["W2i"], dtype=np.float64)
    b2i = np.asarray(inputs["b2i"], dtype=np.float64)
    W1e = np.asarray(inputs["W1e"], dtype=np.float64)
    b1e = np.asarray(inputs["b1e"], dtype=np.float64)
    W2e = np.asarray(inputs["W2e"], dtype=np.float64)
    b2e = np.asarray(inputs["b2e"], dtype=np.float64)

    # ---- host precompute of tiny per-cell scalars (float64 for accuracy) ----
    Q = SC[:, 0].astype(np.float64)
    eta0 = SC[:, 1].astype(np.float64)
    soc_base = SC[:, 3].astype(np.float64)

    feat0 = np.stack(
        [X[:, 0, 1], X[:, 0, 2], X[:, 0, 3], SC[:, 2]], axis=-1
    ).astype(np.float64)  # [B, 4] = (I0, Temp0, U0, R)
    z = feat0 @ W1i.T + b1i
    h0 = _softplus64(z)
    soc_net = (h0 @ W2i.T + b2i)[:, 0]
    soc_init = (soc_base * (1.0 + soc_net)).astype(np.float32)  # [B]

    c = eta0 / (3600.0 * Q)
    b2e_f = float(np.asarray(b2e).reshape(-1)[0])
    w2e_f = float(np.asarray(W2e).reshape(-1)[0])
    q1 = c * (1.0 + b2e_f)  # [B]
    q2 = c * w2e_f          # [B]

    # Stage the eta-net inputs quantization-style: i-stream = w0*I, m-stream
    # = w1*Temp (bf16, scale folded into the cast).  With fold=True, 1/w0
    # moves into q1/q2 so the same i-stream also serves the dt*I product:
    #   incr = dt*I*(q1+q2*h) = dt*(w0*I)*((q1+q2*h)/w0)
    w0 = float(np.asarray(W1e).reshape(-1)[0])
    w1 = float(np.asarray(W1e).reshape(-1)[1])
    b1e_f = float(np.asarray(b1e).reshape(-1)[0])
    fold = abs(w0) > 1e-20
    if fold:
        q1 = q1 / w0
        q2 = q2 / w0
        iscale = w0
    else:
        iscale = 1.0  # raw I stream; pre-act assembled on DVE with w0=0

    P = np.stack(
        [soc_init.astype(np.float64), q1, q2, np.full_like(q1, b1e_f)], axis=-1
    ).astype(np.float32)  # [B, 4]

    tcol = np.ascontiguousarray(X[:, :, 0])                           # [B, T] f32
    icol = (X[:, :, 1] * np.float32(iscale)).astype(ml_dtypes.bfloat16)
    mcol = (X[:, :, 2] * np.float32(w1)).astype(ml_dtypes.bfloat16)

    in_maps = []
    for ci in range(NCORES):
        sl = slice(ci * BS, (ci + 1) * BS)
        in_maps.append(
            {
                "t": tcol[sl],
                "i": np.ascontiguousarray(icol[sl]),
                "m": np.ascontiguousarray(mcol[sl]),
                "p": np.ascontiguousarray(P[sl]),
            }
        )

    return (fold, w0), in_maps, soc_init


def kernel(X, SC, W1i, b1i, W2i, b2i, W1e, b1e, W2e, b2e):
    from concourse.bass_utils import run_bass_kernel_spmd

    params, in_maps, soc_init = _prep(
        dict(X=X, SC=SC, W1i=W1i, b1i=b1i, W2i=W2i, b2i=b2i,
             W1e=W1e, b1e=b1e, W2e=W2e, b2e=b2e)
    )
    nc = _build_program(*params)

    res = run_bass_kernel_spmd(nc, in_maps, list(range(NCORES)))
    out = np.concatenate(
        [res.results[ci]["o"].astype(np.float32) for ci in range(NCORES)], axis=0
    )
    out[:, 0] = soc_init  # device never writes column 0
    return out.reshape(B, T, 1)
